# revision 62
# baseline (speedup 1.0000x reference)
"""MoE layer (top-1 routing) Trainium2 Bass kernel — expert-parallel over 8 cores.

Model (reference): B=4,S=1024,D=512,H=2048,E=8
    logits = x@Wg + bg ; top-1 expert per token ; per-expert FFN
    out[t] = sc[t] * ( relu(x[t]@W1[e] + b1[e]) @ W2[e] + b2[e] ),  e = argmax(logits[t])

Two SPMD launches on 8 cores:
  1. gate:  token-parallel — core k computes fp32 gate logits for tokens
     [512k, 512k+512) with the tokens as the matmul *stationary* operand so the
     logits land directly as [128 tokens, E] in PSUM (no transposes), then
     derives the argmax expert id and softmax top-1 score on device.
     The host only reshuffles the resulting (id, score) pairs into per-expert
     dispatch lists (the all-to-all "dispatch keyed on top-1 index").
  2. ffn:   expert-parallel — the host passes each core its tokens already
     compacted AND transposed ([D, T] fp16, the dispatch half of the
     all-to-all), plus its expert(s) weights in fp16. The FFN runs in fp16
     operands with fp32 PSUM accumulation (rel err ~3e-4 vs 2e-2 tolerance);
     FFN2 produces out^T [D, T]; bias + gate score are fused in one
     scalar_tensor_tensor op per output tile. The host scatters the returned
     compacted columns into the full output (combine).

Load balance: expert token counts are data-dependent (max 622 vs mean 512).
The ffn program processes a fixed set of token "chunks", each bound to one of
up to two weight slots; the host packs overflow tokens of the hottest expert
into the secondary slot of cores whose own expert is cold, so every core does
the same (smaller) amount of matmul work.

kernel(**inputs) takes FULL inputs and returns the FULL (B,S,D) output.
"""
import sys

sys.path.insert(0, "/opt/trn_rl_repo")

import numpy as np

import concourse.bass as bass
import concourse.mybir as mybir
import concourse.tile as tile
from concourse import bacc
from concourse.bass_utils import run_bass_kernel_spmd

F32 = mybir.dt.float32
F16 = mybir.dt.float16

# problem shapes (hardcoded per contest rules)
B, S, D, H, E = 4, 1024, 512, 2048, 8
N = B * S              # 4096 tokens
P = 128                # partitions
DCH = D // P           # 4 contraction chunks over D
HCH = H // P           # 16 chunks over H
NS = N // 8            # 512 tokens per core in the gate launch
NCORES = 8

_CACHED = {}


# ---------------------------------------------------------------------------
# launch 1: distributed gating (token-parallel, fp32)
# ---------------------------------------------------------------------------
def build_gate():
    nc = bacc.Bacc("TRN2", target_bir_lowering=False, debug=False,
                   num_devices=NCORES)
    xt_d = nc.dram_tensor("xt", [D, NS], F32, kind="ExternalInput").ap()
    wg_d = nc.dram_tensor("wg", [D, E], F32, kind="ExternalInput").ap()
    bg_d = nc.dram_tensor("bgr", [1, E], F32, kind="ExternalInput").ap()
    ev_d = nc.dram_tensor("evec", [P, 4 * E], F32, kind="ExternalInput").ap()
    # gout[:, 0:4] = expert id (as f32) ; gout[:, 4:8] = top-1 score.
    # token t = 512*core + 128*j + p lives at [p, j].
    go_d = nc.dram_tensor("gout", [P, 8], F32, kind="ExternalOutput").ap()

    with tile.TileContext(nc) as tc:
        with (
            tc.tile_pool(name="cst", bufs=1) as cst,
            tc.tile_pool(name="ps", bufs=1, space="PSUM") as psp,
            tc.tile_pool(name="sm", bufs=1) as sm,
        ):
            # the 1MB token slice in two halves on the SP queue (half-0's
            # gating pipelines under half-1's DMA); constants ride the
            # Activation queue so they never delay it
            xt_sb = cst.tile([P, DCH, NS], F32, tag="xt")
            xt_r = xt_d.rearrange("(dc p) t -> p dc t", p=P)
            for j in range(4):
                nc.sync.dma_start(xt_sb[:, :, P * j:P * (j + 1)],
                                  xt_r[:, :, P * j:P * (j + 1)])
            wg_sb = cst.tile([P, DCH, E], F32, tag="wg")
            nc.scalar.dma_start(wg_sb[:], wg_d.rearrange("(dc p) e -> p dc e", p=P))
            bg_sb = cst.tile([1, E], F32, tag="bg")
            nc.gpsimd.dma_start(bg_sb[:], bg_d)
            ev_sb = cst.tile([P, 4 * E], F32, tag="ev")
            nc.gpsimd.dma_start(ev_sb[:], ev_d)
            ones = cst.tile([1, P], F32, tag="ones")
            nc.vector.memset(ones[:], 1.0)

            # logits [128 tokens, 4 j, 8 e]: stationary = token chunk; the
            # gate bias rides the PSUM accumulation as a K=1 ones matmul
            nmax = sm.tile([P, 4], F32, tag="nmax")
            zin = sm.tile([P, 4, E], F32, tag="zin")
            z = sm.tile([P, 4, E], F32, tag="z")
            ssum = sm.tile([P, 4], F32, tag="ssum")
            me = sm.tile([P, 4, E], F32, tag="me")
            gout = sm.tile([P, 8], F32, tag="gout")
            for j in range(4):
                lg = psp.tile([P, E], F32, tag=f"lg{j}", name=f"lg{j}")
                for d in range(DCH):
                    nc.tensor.matmul(
                        lg[:], xt_sb[:, d, P * j:P * (j + 1)],
                        wg_sb[:, d, :], start=(d == 0), stop=False)
                nc.tensor.matmul(
                    lg[:], ones[:], bg_sb[:], start=False, stop=True)
                nc.vector.tensor_reduce(
                    nmax[:, j:j + 1], lg[:], axis=mybir.AxisListType.X,
                    op=mybir.AluOpType.max, negate=True)   # = -max
                nc.vector.tensor_scalar_add(
                    zin[:, j, :], lg[:], nmax[:, j:j + 1])
                # score denominator via fused bias+accum: one Act hop
                nc.scalar.activation(
                    z[:, j, :], lg[:],
                    mybir.ActivationFunctionType.Exp,
                    bias=nmax[:, j:j + 1],
                    accum_out=ssum[:, j:j + 1])
                # expert id = sum_e e * [zin_e == 0]
                nc.vector.scalar_tensor_tensor(
                    me[:, j, :], zin[:, j, :], 0.0, ev_sb[:, E * j:E * (j + 1)],
                    op0=mybir.AluOpType.is_equal, op1=mybir.AluOpType.mult)
                nc.vector.tensor_reduce(
                    gout[:, j:j + 1], me[:, j, :], axis=mybir.AxisListType.X,
                    op=mybir.AluOpType.add)
            nc.vector.reciprocal(gout[:, 4:8], ssum[:])
            nc.sync.dma_start(go_d, gout[:])

    nc.compile()
    return nc


# ---------------------------------------------------------------------------
# launch 2: expert FFN (expert-parallel, fp16)
# ---------------------------------------------------------------------------
def build_ffn(chunks, nslots):
    """chunks: list of (slot, t0, t1), t1-t0 <= 320, ordered, t0[0]=0.
    Token columns [t0, t1) are processed with weight slot `slot`."""
    T = chunks[-1][2]
    nc = bacc.Bacc("TRN2", target_bir_lowering=False, debug=False,
                   num_devices=NCORES)
    xt_d = nc.dram_tensor("xt", [D, T], F16, kind="ExternalInput").ap()
    # narrow later chunks get a separate 256-col zero-padded token tensor so
    # their DMA descriptors stay >=512B (narrow descriptors run at half rate)
    aux = [c for c in chunks[1:] if c[2] - c[1] < 256]
    xb_d = (nc.dram_tensor("xtb", [D, 256 * len(aux)], F16,
                           kind="ExternalInput").ap() if aux else None)
    w1_d = [nc.dram_tensor(f"w1_{s}", [D, H], F16, kind="ExternalInput").ap()
            for s in range(nslots)]
    # w2 host-packed d-chunk major, exactly the SBUF layout: [DCH, P, HCH, P]
    w2_d = [nc.dram_tensor(f"w2_{s}", [DCH, P, HCH, P], F16,
                           kind="ExternalInput").ap()
            for s in range(nslots)]
    # all biases bundled in one transfer: per slot HCH cols of b1 then DCH of b2
    bb_d = nc.dram_tensor("biasb", [P, (HCH + DCH) * nslots], F32,
                          kind="ExternalInput").ap()
    sc_d = nc.dram_tensor("scr", [P, T], F16, kind="ExternalInput").ap()
    Tpad = max([T] + [c[1] + 256 for c in aux])
    ho_d = nc.dram_tensor("hout", [D, Tpad], F16, kind="ExternalOutput").ap()
    ho_r = ho_d.rearrange("(dc p) t -> p dc t", p=P)

    with tile.TileContext(nc) as tc:
        with (
            tc.tile_pool(name="cst", bufs=1) as cst,
            tc.tile_pool(name="ps1", bufs=3, space="PSUM") as ps1,
            tc.tile_pool(name="ps2", bufs=1, space="PSUM") as ps2,
            tc.tile_pool(name="outp", bufs=2) as outp,
        ):
            # PE warm-up: dummy matmuls on a zeroed tile keep the tensor
            # engine busy from t~1us so it reaches full p-state before the
            # real FFN1 work arrives (the cost model ramps over 3us)
            warm = cst.tile([P, 512], F16, tag="warm")
            nc.vector.memset(warm[:], 0.0)
            psw = ps1.tile([P, 512], F32, tag="psw", bufs=1)
            for _ in range(8):
                nc.tensor.matmul(psw[:], warm[:, :P], warm[:],
                                 start=True, stop=True)

            # THE ENTIRE input stream rides the single SP (HWDGE) queue in
            # exact consumption order: back-to-back transfers, no SWDGE
            # sequencer bubbles, no cross-queue arbitration gaps.
            # Biases go as one bundled transfer on the Act queue instead.
            xt_sb = cst.tile([P, DCH, Tpad], F16, tag="xt")
            xt_r = xt_d.rearrange("(dc p) t -> p dc t", p=P)
            xb_r = (xb_d.rearrange("(dc p) t -> p dc t", p=P)
                    if aux else None)
            nc.sync.dma_start(xt_sb[:, :, chunks[0][1]:chunks[0][2]],
                              xt_r[:, :, chunks[0][1]:chunks[0][2]])
            bb_sb = cst.tile([P, (HCH + DCH) * nslots], F32, tag="biasb")
            nc.scalar.dma_start(bb_sb[:], bb_d)
            b1_sb = [bb_sb[:, (HCH + DCH) * s:(HCH + DCH) * s + HCH]
                     for s in range(nslots)]
            b2_sb = [bb_sb[:, (HCH + DCH) * s + HCH:(HCH + DCH) * (s + 1)]
                     for s in range(nslots)]
            sc_sb = cst.tile([P, T], F16, tag="scr")

            w1_sb = [cst.tile([P, DCH, H], F16, tag=f"w1_{s}", name=f"w1_{s}")
                     for s in range(nslots)]
            w2_sb = [cst.tile([P, DCH, HCH, P], F16, tag=f"w2_{s}",
                              name=f"w2_{s}")
                     for s in range(nslots)]
            w1_r = [w1_d[s].rearrange("(dc p) h -> p dc h", p=P)
                    for s in range(nslots)]
            if nslots == 1:
                w1_order = [(0, 0, 2), (0, 2, 4), (0, 4, 8), (0, 8, 12),
                            (0, 12, 16)]
            else:
                w1_order = [(0, 0, 2), (0, 2, 4), (0, 4, 8), (1, 0, 4),
                            (0, 8, 12), (1, 4, 8), (0, 12, 16), (1, 8, 12),
                            (1, 12, 16)]
            for pi, (s, h0, h1_) in enumerate(w1_order):
                nc.sync.dma_start(
                    w1_sb[s][:, :, P * h0:P * h1_], w1_r[s][:, :, P * h0:P * h1_])
                if pi == min(2, len(w1_order) - 1):
                    # later chunks' tokens ride the stream here: late enough
                    # not to delay the critical early w1 pieces, early enough
                    # to land before their first FFN1 block
                    ai = 0
                    for _, t0, t1 in chunks[1:]:
                        if t1 - t0 < 256:
                            nc.sync.dma_start(
                                xt_sb[:, :, t0:t0 + 256],
                                xb_r[:, :, 256 * ai:256 * (ai + 1)])
                            ai += 1
                        else:
                            nc.sync.dma_start(xt_sb[:, :, t0:t1],
                                              xt_r[:, :, t0:t1])
            for dd in range(DCH):
                for s in range(nslots):
                    nc.sync.dma_start(w2_sb[s][:, dd], w2_d[s][dd])
                if dd == 0:
                    # score row: lands before the first FFN2 epilogue needs
                    # it, after the critical w1 stream
                    nc.sync.dma_start(sc_sb[:], sc_d)

            # FFN1: h1[h, t] = relu(sum_d W1[d,h] xT[d,t] + b1[h])   (fp16 out)
            # h-blocks processed in the exact order the w1 pieces arrive so
            # the PE tracks the DMA stream without stalls
            h1 = cst.tile([P, HCH, T], F16, tag="h1")
            for s, h0, h1_ in w1_order:
                for ci, (cs, t0, t1) in enumerate(chunks):
                    if cs != s:
                        continue
                    for h in range(h0, h1_):
                        psh = ps1.tile([P, 320], F32, tag="psh")
                        for d in range(DCH):
                            nc.tensor.matmul(
                                psh[:, :t1 - t0],
                                w1_sb[s][:, d, P * h:P * (h + 1)],
                                xt_sb[:, d, t0:t1],
                                start=(d == 0), stop=(d == DCH - 1))
                        # alternate bias+relu between Act and DVE so neither
                        # engine lags the PE's h-block rate
                        if h % 2 == 0:
                            nc.scalar.activation(
                                h1[:, h, t0:t1], psh[:, :t1 - t0],
                                mybir.ActivationFunctionType.Relu,
                                bias=b1_sb[s][:, h:h + 1])
                        else:
                            nc.vector.tensor_scalar(
                                h1[:, h, t0:t1], psh[:, :t1 - t0],
                                b1_sb[s][:, h:h + 1], 0.0,
                                op0=mybir.AluOpType.add,
                                op1=mybir.AluOpType.max)

            # FFN2 (transposed): out[d, t] = (sum_k h1[k,t] W2[k,d] + b2[d]) * sc[t]
            # one sub-round per output d-chunk; epilogue + out DMA of sub-round
            # dd overlap the matmuls of dd+1
            for dd in range(DCH):
                # per-chunk k-loops: the big chunk's epilogue + out DMA
                # overlap the small chunk's matmuls, shrinking the tail
                for ci, (s, t0, t1) in enumerate(chunks):
                    po = ps2.tile([P, 320], F32, tag=f"po{dd % 2}_{ci}",
                                  name=f"po{dd}_{ci}")
                    for k in range(HCH):
                        nc.tensor.matmul(
                            po[:, :t1 - t0],
                            w2_sb[s][:, dd, k, :],
                            h1[:, k, t0:t1],
                            start=(k == 0), stop=(k == HCH - 1))
                    osb = outp.tile([P, 320], F16, tag=f"osb{dd % 2}_{ci}")
                    nc.vector.scalar_tensor_tensor(
                        osb[:, :t1 - t0], po[:, :t1 - t0],
                        b2_sb[s][:, dd:dd + 1], sc_sb[:, t0:t1],
                        op0=mybir.AluOpType.add, op1=mybir.AluOpType.mult)
                    w = 256 if t1 - t0 < 256 and t0 + 256 <= Tpad else t1 - t0
                    nc.scalar.dma_start(ho_r[:, dd, t0:t0 + w], osb[:, :w])

    nc.compile()
    return nc


# ---------------------------------------------------------------------------
# host driver
# ---------------------------------------------------------------------------
def _nc_gate():
    if "gate" not in _CACHED:
        _CACHED["gate"] = build_gate()
    return _CACHED["gate"]


def _nc_ffn(chunks, nslots):
    key = ("ffn", tuple(chunks), nslots)
    if key not in _CACHED:
        _CACHED[key] = build_ffn(chunks, nslots)
    _CACHED["ffn"] = _CACHED[key]
    return _CACHED[key]


def gate_in_maps(xf, Wg, bg):
    evec = np.tile(np.arange(E, dtype=np.float32), (P, 4)).astype(np.float32)
    bgr = np.ascontiguousarray(bg.reshape(1, E).astype(np.float32))
    maps = []
    for k in range(NCORES):
        maps.append(dict(
            xt=np.ascontiguousarray(xf[NS * k:NS * (k + 1)].T),
            wg=Wg, bgr=bgr, evec=evec,
        ))
    return maps


def plan_schedule(counts):
    """Choose (chunks, nslots, assign) for the observed per-expert counts.
    assign: per core, ordered list of (expert, chunk_index, n_tokens).

    Balanced template (T=544): cores 0..5 run one 'middle' expert in both
    chunks (cap 320+224); the heaviest expert is split over the A-chunks
    (320 each) of cores 6,7 whose B-chunks (224 each) take the lightest."""
    order = np.argsort(-counts)          # experts, heaviest first
    c = counts[order]
    if c[0] <= 624 and c[1] <= 532 and c[7] <= 440:
        chunks = [(0, 0, 312), (1, 312, 532)]
        assign = []
        for i in range(6):               # middle experts: solo core
            e = int(order[i + 1])
            n = int(counts[e])
            assign.append([(e, 0, min(n, 312)), (e, 1, max(0, n - 312))])
        eh, el = int(order[0]), int(order[7])
        nh, nl = int(counts[eh]), int(counts[el])
        h0, l0 = (nh + 1) // 2, (nl + 1) // 2
        assign.append([(eh, 0, h0), (el, 1, l0)])
        assign.append([(eh, 0, nh - h0), (el, 1, nl - l0)])
        return chunks, 2, assign
    # fallback: one expert per core, capacity = max count rounded up
    cap = int(-(-counts.max() // 64) * 64)
    chunks = [(0, lo, min(lo + 320, cap)) for lo in range(0, cap, 320)]
    assign = []
    for e in range(E):
        n = int(counts[e])
        segs = []
        for ci, (_, t0, t1) in enumerate(chunks):
            segs.append((e, ci, max(0, min(n, t1) - t0)))
        assign.append(segs)
    return chunks, 1, assign


def ffn_in_maps(xf, W1, b1, W2, b2, ids_all, sc_all, chunks, nslots, assign):
    T = chunks[-1][2]
    maps = []
    offs = [c[1] for c in chunks]
    pos = {e: 0 for e in range(E)}       # global per-expert cursor
    for core in range(NCORES):
        segs = assign[core]
        xt = np.zeros((T, D), dtype=np.float16)
        scr = np.zeros(T, dtype=np.float32)
        slot_exp = [None] * nslots
        for e, ci, n in segs:
            slot_exp[chunks[ci][0]] = e
            if n == 0:
                continue
            t0 = offs[ci]
            rows = ids_all[e][pos[e]:pos[e] + n]
            xt[t0:t0 + n] = xf[rows].astype(np.float16)
            scr[t0:t0 + n] = sc_all[rows]
            pos[e] += n
        m = dict(
            xt=np.ascontiguousarray(xt.T),
            scr=np.ascontiguousarray(np.tile(scr[None, :].astype(np.float16), (P, 1))),
        )
        aux = [c for c in chunks[1:] if c[2] - c[1] < 256]
        if aux:
            xtb = np.zeros((256 * len(aux), D), dtype=np.float16)
            for ai, (_, t0, t1) in enumerate(aux):
                xtb[256 * ai:256 * ai + (t1 - t0)] = xt[t0:t1]
            m["xtb"] = np.ascontiguousarray(xtb.T)
        biasb = np.zeros((P, (HCH + DCH) * nslots), dtype=np.float32)
        for s in range(nslots):
            e = slot_exp[s] if slot_exp[s] is not None else 0
            m[f"w1_{s}"] = np.ascontiguousarray(W1[e].astype(np.float16))
            # [H, D] -> [DCH, P(k), HCH, P(d)] (the ffn program's SBUF layout)
            m[f"w2_{s}"] = np.ascontiguousarray(
                W2[e].astype(np.float16).reshape(HCH, P, DCH, P)
                .transpose(2, 1, 0, 3))
            o = (HCH + DCH) * s
            biasb[:, o:o + HCH] = b1[e].reshape(HCH, P).T
            biasb[:, o + HCH:o + HCH + DCH] = b2[e].reshape(DCH, P).T
        m["biasb"] = biasb
        maps.append(m)
    return maps


def kernel(x, Wg, bg, W1, b1, W2, b2):
    x = np.ascontiguousarray(np.asarray(x, dtype=np.float32))
    Wg = np.ascontiguousarray(np.asarray(Wg, dtype=np.float32))
    bg = np.ascontiguousarray(np.asarray(bg, dtype=np.float32))
    W1 = np.ascontiguousarray(np.asarray(W1, dtype=np.float32))
    b1 = np.ascontiguousarray(np.asarray(b1, dtype=np.float32))
    W2 = np.ascontiguousarray(np.asarray(W2, dtype=np.float32))
    b2 = np.ascontiguousarray(np.asarray(b2, dtype=np.float32))
    xf = x.reshape(N, D)

    res1 = run_bass_kernel_spmd(
        _nc_gate(), gate_in_maps(xf, Wg, bg), core_ids=list(range(NCORES)))
    eid = np.zeros(N, dtype=np.int64)
    sc_all = np.zeros(N, dtype=np.float32)
    for k in range(NCORES):
        g = res1.results[k]["gout"]
        # [p, j] -> token 512k + 128j + p
        eid[NS * k:NS * (k + 1)] = np.rint(g[:, 0:4].T.reshape(-1)).astype(np.int64)
        sc_all[NS * k:NS * (k + 1)] = g[:, 4:8].T.reshape(-1)

    ids_all = [np.nonzero(eid == c)[0] for c in range(E)]
    counts = np.array([len(i) for i in ids_all])
    chunks, nslots, assign = plan_schedule(counts)
    res2 = run_bass_kernel_spmd(
        _nc_ffn(chunks, nslots),
        ffn_in_maps(xf, W1, b1, W2, b2, ids_all, sc_all, chunks, nslots, assign),
        core_ids=list(range(NCORES)))

    out = np.zeros((N, D), dtype=np.float32)
    offs = [c[1] for c in chunks]
    pos = {e: 0 for e in range(E)}
    for core in range(NCORES):
        ot = res2.results[core]["hout"].T.astype(np.float32)   # [T, D]
        for e, ci, n in assign[core]:
            if n == 0:
                continue
            t0 = offs[ci]
            rows = ids_all[e][pos[e]:pos[e] + n]
            out[rows] = ot[t0:t0 + n]
            pos[e] += n
    return out.reshape(B, S, D)


def run_traced(np_inputs, **kw):
    raise NotImplementedError("use perf.py (TimelineSim) for timing")


# revision 63
# speedup vs baseline: 1.0083x; 1.0083x over previous
"""MoE layer (top-1 routing) Trainium2 Bass kernel — expert-parallel over 8 cores.

Model (reference): B=4,S=1024,D=512,H=2048,E=8
    logits = x@Wg + bg ; top-1 expert per token ; per-expert FFN
    out[t] = sc[t] * ( relu(x[t]@W1[e] + b1[e]) @ W2[e] + b2[e] ),  e = argmax(logits[t])

Two SPMD launches on 8 cores:
  1. gate:  token-parallel — core k computes fp32 gate logits for tokens
     [512k, 512k+512) with the tokens as the matmul *stationary* operand so the
     logits land directly as [128 tokens, E] in PSUM (no transposes), then
     derives the argmax expert id and softmax top-1 score on device.
     The host only reshuffles the resulting (id, score) pairs into per-expert
     dispatch lists (the all-to-all "dispatch keyed on top-1 index").
  2. ffn:   expert-parallel — the host passes each core its tokens already
     compacted AND transposed ([D, T] fp16, the dispatch half of the
     all-to-all), plus its expert(s) weights in fp16. The FFN runs in fp16
     operands with fp32 PSUM accumulation (rel err ~3e-4 vs 2e-2 tolerance);
     FFN2 produces out^T [D, T]; bias + gate score are fused in one
     scalar_tensor_tensor op per output tile. The host scatters the returned
     compacted columns into the full output (combine).

Load balance: expert token counts are data-dependent (max 622 vs mean 512).
The ffn program processes a fixed set of token "chunks", each bound to one of
up to two weight slots; the host packs overflow tokens of the hottest expert
into the secondary slot of cores whose own expert is cold, so every core does
the same (smaller) amount of matmul work.

kernel(**inputs) takes FULL inputs and returns the FULL (B,S,D) output.
"""
import sys

sys.path.insert(0, "/opt/trn_rl_repo")

import numpy as np

import concourse.bass as bass
import concourse.mybir as mybir
import concourse.tile as tile
from concourse import bacc
from concourse.bass_utils import run_bass_kernel_spmd

F32 = mybir.dt.float32
F16 = mybir.dt.float16

# problem shapes (hardcoded per contest rules)
B, S, D, H, E = 4, 1024, 512, 2048, 8
N = B * S              # 4096 tokens
P = 128                # partitions
DCH = D // P           # 4 contraction chunks over D
HCH = H // P           # 16 chunks over H
NS = N // 8            # 512 tokens per core in the gate launch
NCORES = 8

_CACHED = {}


# ---------------------------------------------------------------------------
# launch 1: distributed gating (token-parallel, fp32)
# ---------------------------------------------------------------------------
def build_gate():
    nc = bacc.Bacc("TRN2", target_bir_lowering=False, debug=False,
                   num_devices=NCORES)
    xt_d = nc.dram_tensor("xt", [D, NS], F32, kind="ExternalInput").ap()
    wg_d = nc.dram_tensor("wg", [D, E], F32, kind="ExternalInput").ap()
    bg_d = nc.dram_tensor("bgr", [1, E], F32, kind="ExternalInput").ap()
    ev_d = nc.dram_tensor("evec", [P, 4 * E], F32, kind="ExternalInput").ap()
    # gout[:, 0:4] = expert id (as f32) ; gout[:, 4:8] = top-1 score.
    # token t = 512*core + 128*j + p lives at [p, j].
    go_d = nc.dram_tensor("gout", [P, 8], F32, kind="ExternalOutput").ap()

    with tile.TileContext(nc) as tc:
        with (
            tc.tile_pool(name="cst", bufs=1) as cst,
            tc.tile_pool(name="ps", bufs=1, space="PSUM") as psp,
            tc.tile_pool(name="sm", bufs=1) as sm,
        ):
            # the 1MB token slice in two halves on the SP queue (half-0's
            # gating pipelines under half-1's DMA); constants ride the
            # Activation queue so they never delay it
            xt_sb = cst.tile([P, DCH, NS], F32, tag="xt")
            xt_r = xt_d.rearrange("(dc p) t -> p dc t", p=P)
            for j in range(4):
                nc.sync.dma_start(xt_sb[:, :, P * j:P * (j + 1)],
                                  xt_r[:, :, P * j:P * (j + 1)])
            wg_sb = cst.tile([P, DCH, E], F32, tag="wg")
            nc.scalar.dma_start(wg_sb[:], wg_d.rearrange("(dc p) e -> p dc e", p=P))
            bg_sb = cst.tile([1, E], F32, tag="bg")
            nc.gpsimd.dma_start(bg_sb[:], bg_d)
            ev_sb = cst.tile([P, 4 * E], F32, tag="ev")
            nc.gpsimd.dma_start(ev_sb[:], ev_d)
            ones = cst.tile([1, P], F32, tag="ones")
            nc.vector.memset(ones[:], 1.0)

            # logits [128 tokens, 4 j, 8 e]: stationary = token chunk; the
            # gate bias rides the PSUM accumulation as a K=1 ones matmul
            nmax = sm.tile([P, 4], F32, tag="nmax")
            zin = sm.tile([P, 4, E], F32, tag="zin")
            z = sm.tile([P, 4, E], F32, tag="z")
            ssum = sm.tile([P, 4], F32, tag="ssum")
            me = sm.tile([P, 4, E], F32, tag="me")
            gout = sm.tile([P, 8], F32, tag="gout")
            for j in range(4):
                lg = psp.tile([P, E], F32, tag=f"lg{j}", name=f"lg{j}")
                for d in range(DCH):
                    nc.tensor.matmul(
                        lg[:], xt_sb[:, d, P * j:P * (j + 1)],
                        wg_sb[:, d, :], start=(d == 0), stop=False)
                nc.tensor.matmul(
                    lg[:], ones[:], bg_sb[:], start=False, stop=True)
                nc.vector.tensor_reduce(
                    nmax[:, j:j + 1], lg[:], axis=mybir.AxisListType.X,
                    op=mybir.AluOpType.max, negate=True)   # = -max
                nc.vector.tensor_scalar_add(
                    zin[:, j, :], lg[:], nmax[:, j:j + 1])
                # score denominator via fused bias+accum: one Act hop
                nc.scalar.activation(
                    z[:, j, :], lg[:],
                    mybir.ActivationFunctionType.Exp,
                    bias=nmax[:, j:j + 1],
                    accum_out=ssum[:, j:j + 1])
                # expert id = sum_e e * [zin_e == 0]
                nc.vector.scalar_tensor_tensor(
                    me[:, j, :], zin[:, j, :], 0.0, ev_sb[:, E * j:E * (j + 1)],
                    op0=mybir.AluOpType.is_equal, op1=mybir.AluOpType.mult)
                nc.vector.tensor_reduce(
                    gout[:, j:j + 1], me[:, j, :], axis=mybir.AxisListType.X,
                    op=mybir.AluOpType.add)
            nc.vector.reciprocal(gout[:, 4:8], ssum[:])
            nc.sync.dma_start(go_d, gout[:])

    nc.compile()
    return nc


# ---------------------------------------------------------------------------
# launch 2: expert FFN (expert-parallel, fp16)
# ---------------------------------------------------------------------------
def build_ffn(chunks, nslots):
    """chunks: list of (slot, t0, t1), t1-t0 <= 320, ordered, t0[0]=0.
    Token columns [t0, t1) are processed with weight slot `slot`."""
    T = chunks[-1][2]
    nc = bacc.Bacc("TRN2", target_bir_lowering=False, debug=False,
                   num_devices=NCORES)
    xt_d = nc.dram_tensor("xt", [D, T], F16, kind="ExternalInput").ap()
    # narrow later chunks get a separate 256-col zero-padded token tensor so
    # their DMA descriptors stay >=512B (narrow descriptors run at half rate)
    aux = [c for c in chunks[1:] if c[2] - c[1] < 256]
    xb_d = (nc.dram_tensor("xtb", [D, 256 * len(aux)], F16,
                           kind="ExternalInput").ap() if aux else None)
    w1_d = [nc.dram_tensor(f"w1_{s}", [D, H], F16, kind="ExternalInput").ap()
            for s in range(nslots)]
    # w2 host-packed d-chunk major, exactly the SBUF layout: [DCH, P, HCH, P]
    w2_d = [nc.dram_tensor(f"w2_{s}", [DCH, P, HCH, P], F16,
                           kind="ExternalInput").ap()
            for s in range(nslots)]
    # all biases bundled in one transfer: per slot HCH cols of b1 then DCH of b2
    bb_d = nc.dram_tensor("biasb", [P, (HCH + DCH) * nslots], F32,
                          kind="ExternalInput").ap()
    sc_d = nc.dram_tensor("scr", [P, T], F16, kind="ExternalInput").ap()
    Tpad = max([T] + [c[1] + 256 for c in aux])
    ho_d = nc.dram_tensor("hout", [D, Tpad], F16, kind="ExternalOutput").ap()
    ho_r = ho_d.rearrange("(dc p) t -> p dc t", p=P)

    with tile.TileContext(nc) as tc:
        with (
            tc.tile_pool(name="cst", bufs=1) as cst,
            tc.tile_pool(name="ps1", bufs=4, space="PSUM") as ps1,
            tc.tile_pool(name="ps2", bufs=1, space="PSUM") as ps2,
            tc.tile_pool(name="outp", bufs=2) as outp,
        ):
            # PE warm-up: dummy matmuls on a zeroed tile keep the tensor
            # engine busy from t~1us so it reaches full p-state before the
            # real FFN1 work arrives (the cost model ramps over 3us)
            warm = cst.tile([P, 512], F16, tag="warm")
            nc.vector.memset(warm[:], 0.0)
            psw = ps2.tile([P, 320], F32, tag="po0_0", name="psw")
            for _ in range(11):
                nc.tensor.matmul(psw[:], warm[:, :P], warm[:, :320],
                                 start=True, stop=True)

            # THE ENTIRE input stream rides the single SP (HWDGE) queue in
            # exact consumption order: back-to-back transfers, no SWDGE
            # sequencer bubbles, no cross-queue arbitration gaps.
            # Biases go as one bundled transfer on the Act queue instead.
            xt_sb = cst.tile([P, DCH, Tpad], F16, tag="xt")
            xt_r = xt_d.rearrange("(dc p) t -> p dc t", p=P)
            xb_r = (xb_d.rearrange("(dc p) t -> p dc t", p=P)
                    if aux else None)
            nc.sync.dma_start(xt_sb[:, :, chunks[0][1]:chunks[0][2]],
                              xt_r[:, :, chunks[0][1]:chunks[0][2]])
            bb_sb = cst.tile([P, (HCH + DCH) * nslots], F32, tag="biasb")
            nc.scalar.dma_start(bb_sb[:], bb_d)
            b1_sb = [bb_sb[:, (HCH + DCH) * s:(HCH + DCH) * s + HCH]
                     for s in range(nslots)]
            b2_sb = [bb_sb[:, (HCH + DCH) * s + HCH:(HCH + DCH) * (s + 1)]
                     for s in range(nslots)]
            sc_sb = cst.tile([P, T], F16, tag="scr")

            w1_sb = [cst.tile([P, DCH, H], F16, tag=f"w1_{s}", name=f"w1_{s}")
                     for s in range(nslots)]
            w2_sb = [cst.tile([P, DCH, HCH, P], F16, tag=f"w2_{s}",
                              name=f"w2_{s}")
                     for s in range(nslots)]
            w1_r = [w1_d[s].rearrange("(dc p) h -> p dc h", p=P)
                    for s in range(nslots)]
            if nslots == 1:
                w1_order = [(0, 0, 2), (0, 2, 4), (0, 4, 8), (0, 8, 12),
                            (0, 12, 16)]
            else:
                w1_order = [(0, 0, 2), (0, 2, 4), (0, 4, 8), (1, 0, 4),
                            (0, 8, 12), (1, 4, 8), (0, 12, 16), (1, 8, 12),
                            (1, 12, 16)]
            for pi, (s, h0, h1_) in enumerate(w1_order):
                nc.sync.dma_start(
                    w1_sb[s][:, :, P * h0:P * h1_], w1_r[s][:, :, P * h0:P * h1_])
                if pi == min(2, len(w1_order) - 1):
                    # later chunks' tokens ride the stream here: late enough
                    # not to delay the critical early w1 pieces, early enough
                    # to land before their first FFN1 block
                    ai = 0
                    for _, t0, t1 in chunks[1:]:
                        if t1 - t0 < 256:
                            nc.sync.dma_start(
                                xt_sb[:, :, t0:t0 + 256],
                                xb_r[:, :, 256 * ai:256 * (ai + 1)])
                            ai += 1
                        else:
                            nc.sync.dma_start(xt_sb[:, :, t0:t1],
                                              xt_r[:, :, t0:t1])
            for dd in range(DCH):
                for s in range(nslots):
                    nc.sync.dma_start(w2_sb[s][:, dd], w2_d[s][dd])
                if dd == 0:
                    # score row: lands before the first FFN2 epilogue needs
                    # it, after the critical w1 stream
                    nc.sync.dma_start(sc_sb[:], sc_d)

            # FFN1: h1[h, t] = relu(sum_d W1[d,h] xT[d,t] + b1[h])   (fp16 out)
            # h-blocks processed in the exact order the w1 pieces arrive so
            # the PE tracks the DMA stream without stalls
            h1 = cst.tile([P, HCH, T], F16, tag="h1")
            for s, h0, h1_ in w1_order:
                for ci, (cs, t0, t1) in enumerate(chunks):
                    if cs != s:
                        continue
                    for h in range(h0, h1_):
                        psh = ps1.tile([P, 320], F32, tag="psh")
                        for d in range(DCH):
                            nc.tensor.matmul(
                                psh[:, :t1 - t0],
                                w1_sb[s][:, d, P * h:P * (h + 1)],
                                xt_sb[:, d, t0:t1],
                                start=(d == 0), stop=(d == DCH - 1))
                        # alternate bias+relu between Act and DVE so neither
                        # engine lags the PE's h-block rate
                        if h % 2 == 0:
                            nc.scalar.activation(
                                h1[:, h, t0:t1], psh[:, :t1 - t0],
                                mybir.ActivationFunctionType.Relu,
                                bias=b1_sb[s][:, h:h + 1])
                        else:
                            nc.vector.tensor_scalar(
                                h1[:, h, t0:t1], psh[:, :t1 - t0],
                                b1_sb[s][:, h:h + 1], 0.0,
                                op0=mybir.AluOpType.add,
                                op1=mybir.AluOpType.max)

            # FFN2 (transposed): out[d, t] = (sum_k h1[k,t] W2[k,d] + b2[d]) * sc[t]
            # one sub-round per output d-chunk; epilogue + out DMA of sub-round
            # dd overlap the matmuls of dd+1
            for dd in range(DCH):
                # per-chunk k-loops: the big chunk's epilogue + out DMA
                # overlap the small chunk's matmuls, shrinking the tail
                for ci, (s, t0, t1) in enumerate(chunks):
                    po = ps2.tile([P, 320], F32, tag=f"po{dd % 2}_{ci}",
                                  name=f"po{dd}_{ci}")
                    for k in range(HCH):
                        nc.tensor.matmul(
                            po[:, :t1 - t0],
                            w2_sb[s][:, dd, k, :],
                            h1[:, k, t0:t1],
                            start=(k == 0), stop=(k == HCH - 1))
                    osb = outp.tile([P, 320], F16, tag=f"osb{dd % 2}_{ci}")
                    nc.vector.scalar_tensor_tensor(
                        osb[:, :t1 - t0], po[:, :t1 - t0],
                        b2_sb[s][:, dd:dd + 1], sc_sb[:, t0:t1],
                        op0=mybir.AluOpType.add, op1=mybir.AluOpType.mult)
                    w = 256 if t1 - t0 < 256 and t0 + 256 <= Tpad else t1 - t0
                    nc.scalar.dma_start(ho_r[:, dd, t0:t0 + w], osb[:, :w])

    nc.compile()
    return nc


# ---------------------------------------------------------------------------
# host driver
# ---------------------------------------------------------------------------
def _nc_gate():
    if "gate" not in _CACHED:
        _CACHED["gate"] = build_gate()
    return _CACHED["gate"]


def _nc_ffn(chunks, nslots):
    key = ("ffn", tuple(chunks), nslots)
    if key not in _CACHED:
        _CACHED[key] = build_ffn(chunks, nslots)
    _CACHED["ffn"] = _CACHED[key]
    return _CACHED[key]


def gate_in_maps(xf, Wg, bg):
    evec = np.tile(np.arange(E, dtype=np.float32), (P, 4)).astype(np.float32)
    bgr = np.ascontiguousarray(bg.reshape(1, E).astype(np.float32))
    maps = []
    for k in range(NCORES):
        maps.append(dict(
            xt=np.ascontiguousarray(xf[NS * k:NS * (k + 1)].T),
            wg=Wg, bgr=bgr, evec=evec,
        ))
    return maps


def plan_schedule(counts):
    """Choose (chunks, nslots, assign) for the observed per-expert counts.
    assign: per core, ordered list of (expert, chunk_index, n_tokens).

    Balanced template (T=544): cores 0..5 run one 'middle' expert in both
    chunks (cap 320+224); the heaviest expert is split over the A-chunks
    (320 each) of cores 6,7 whose B-chunks (224 each) take the lightest."""
    order = np.argsort(-counts)          # experts, heaviest first
    c = counts[order]
    if c[0] <= 624 and c[1] <= 532 and c[7] <= 440:
        chunks = [(0, 0, 312), (1, 312, 532)]
        assign = []
        for i in range(6):               # middle experts: solo core
            e = int(order[i + 1])
            n = int(counts[e])
            assign.append([(e, 0, min(n, 312)), (e, 1, max(0, n - 312))])
        eh, el = int(order[0]), int(order[7])
        nh, nl = int(counts[eh]), int(counts[el])
        h0, l0 = (nh + 1) // 2, (nl + 1) // 2
        assign.append([(eh, 0, h0), (el, 1, l0)])
        assign.append([(eh, 0, nh - h0), (el, 1, nl - l0)])
        return chunks, 2, assign
    # fallback: one expert per core, capacity = max count rounded up
    cap = int(-(-counts.max() // 64) * 64)
    chunks = [(0, lo, min(lo + 320, cap)) for lo in range(0, cap, 320)]
    assign = []
    for e in range(E):
        n = int(counts[e])
        segs = []
        for ci, (_, t0, t1) in enumerate(chunks):
            segs.append((e, ci, max(0, min(n, t1) - t0)))
        assign.append(segs)
    return chunks, 1, assign


def ffn_in_maps(xf, W1, b1, W2, b2, ids_all, sc_all, chunks, nslots, assign):
    T = chunks[-1][2]
    maps = []
    offs = [c[1] for c in chunks]
    pos = {e: 0 for e in range(E)}       # global per-expert cursor
    for core in range(NCORES):
        segs = assign[core]
        xt = np.zeros((T, D), dtype=np.float16)
        scr = np.zeros(T, dtype=np.float32)
        slot_exp = [None] * nslots
        for e, ci, n in segs:
            slot_exp[chunks[ci][0]] = e
            if n == 0:
                continue
            t0 = offs[ci]
            rows = ids_all[e][pos[e]:pos[e] + n]
            xt[t0:t0 + n] = xf[rows].astype(np.float16)
            scr[t0:t0 + n] = sc_all[rows]
            pos[e] += n
        m = dict(
            xt=np.ascontiguousarray(xt.T),
            scr=np.ascontiguousarray(np.tile(scr[None, :].astype(np.float16), (P, 1))),
        )
        aux = [c for c in chunks[1:] if c[2] - c[1] < 256]
        if aux:
            xtb = np.zeros((256 * len(aux), D), dtype=np.float16)
            for ai, (_, t0, t1) in enumerate(aux):
                xtb[256 * ai:256 * ai + (t1 - t0)] = xt[t0:t1]
            m["xtb"] = np.ascontiguousarray(xtb.T)
        biasb = np.zeros((P, (HCH + DCH) * nslots), dtype=np.float32)
        for s in range(nslots):
            e = slot_exp[s] if slot_exp[s] is not None else 0
            m[f"w1_{s}"] = np.ascontiguousarray(W1[e].astype(np.float16))
            # [H, D] -> [DCH, P(k), HCH, P(d)] (the ffn program's SBUF layout)
            m[f"w2_{s}"] = np.ascontiguousarray(
                W2[e].astype(np.float16).reshape(HCH, P, DCH, P)
                .transpose(2, 1, 0, 3))
            o = (HCH + DCH) * s
            biasb[:, o:o + HCH] = b1[e].reshape(HCH, P).T
            biasb[:, o + HCH:o + HCH + DCH] = b2[e].reshape(DCH, P).T
        m["biasb"] = biasb
        maps.append(m)
    return maps


def kernel(x, Wg, bg, W1, b1, W2, b2):
    x = np.ascontiguousarray(np.asarray(x, dtype=np.float32))
    Wg = np.ascontiguousarray(np.asarray(Wg, dtype=np.float32))
    bg = np.ascontiguousarray(np.asarray(bg, dtype=np.float32))
    W1 = np.ascontiguousarray(np.asarray(W1, dtype=np.float32))
    b1 = np.ascontiguousarray(np.asarray(b1, dtype=np.float32))
    W2 = np.ascontiguousarray(np.asarray(W2, dtype=np.float32))
    b2 = np.ascontiguousarray(np.asarray(b2, dtype=np.float32))
    xf = x.reshape(N, D)

    res1 = run_bass_kernel_spmd(
        _nc_gate(), gate_in_maps(xf, Wg, bg), core_ids=list(range(NCORES)))
    eid = np.zeros(N, dtype=np.int64)
    sc_all = np.zeros(N, dtype=np.float32)
    for k in range(NCORES):
        g = res1.results[k]["gout"]
        # [p, j] -> token 512k + 128j + p
        eid[NS * k:NS * (k + 1)] = np.rint(g[:, 0:4].T.reshape(-1)).astype(np.int64)
        sc_all[NS * k:NS * (k + 1)] = g[:, 4:8].T.reshape(-1)

    ids_all = [np.nonzero(eid == c)[0] for c in range(E)]
    counts = np.array([len(i) for i in ids_all])
    chunks, nslots, assign = plan_schedule(counts)
    res2 = run_bass_kernel_spmd(
        _nc_ffn(chunks, nslots),
        ffn_in_maps(xf, W1, b1, W2, b2, ids_all, sc_all, chunks, nslots, assign),
        core_ids=list(range(NCORES)))

    out = np.zeros((N, D), dtype=np.float32)
    offs = [c[1] for c in chunks]
    pos = {e: 0 for e in range(E)}
    for core in range(NCORES):
        ot = res2.results[core]["hout"].T.astype(np.float32)   # [T, D]
        for e, ci, n in assign[core]:
            if n == 0:
                continue
            t0 = offs[ci]
            rows = ids_all[e][pos[e]:pos[e] + n]
            out[rows] = ot[t0:t0 + n]
            pos[e] += n
    return out.reshape(B, S, D)


def run_traced(np_inputs, **kw):
    raise NotImplementedError("use perf.py (TimelineSim) for timing")
